# revision 24
# baseline (speedup 1.0000x reference)
# Trainium2 Bass kernel for nn_GTEProgramClassification (GNN message passing).
#
# Math (see problem reference):
#   feat_src = mean_s emb[token_id[:, s]]              [N_src, D]
#   msgs     = feat_src[neigh_idx]                     [N_dst, max_deg, D]
#   h        = GRU scan over msgs[:, :deg-1]           (per-node step count)
#   ft       = deg==1 ? msgs[:,0] : LayerNorm(h)
#   out      = ft @ wc.T + bc                          [N_dst, n_classes]
#
# Strategy (8 cores, data-parallel over dst nodes):
#   * Host sorts dst nodes by degree (descending) and deals them across the
#     8 cores so every core sees the exact same degree profile (classes are
#     padded to a multiple of 16 with fake nodes). One SPMD program.
#   * At GRU step t the active nodes are a shrinking prefix of the sorted
#     columns; retired columns keep their final h in place.
#   * Message features come from bf16 dma_gathers of the four subtoken
#     embedding rows (host passes emb pre-cast to bf16; int16 indices
#     interleaved so one node's 4 rows land on the SAME partition in 4
#     adjacent 128-element slots).  Subtoken sum = 3 strided bf16 DVE adds
#     (cheap, frees the gather buffer fast); the row->column transpose is
#     one PE transpose-mode op per 128 nodes (identity stationary).  The
#     1/4 mean scale is folded into w_ih on the host (deg-1 nodes apply it
#     explicitly).
#   * Gathers round-robin over all 4 SWDGE queues: descriptor generation for
#     queue q runs on Q7 core pair (2q, 2q+1), so four gathers generate
#     descriptors concurrently (~4x the single-queue gather throughput).
#   * Layout B on chip: tiles are [D=128 partitions, nodes free].  LayerNorm
#     reductions over D are K=128 matmuls with a 1/128 ones vector;
#     per-node scalars are partition-broadcast via K=1 matmuls.
#   * GRU h/weights in float32r (1 cycle/row); x-side weights bf16.

import numpy as np
import ml_dtypes

import concourse.bacc as bacc
import concourse.bass as bass
import concourse.mybir as mybir
import concourse.tile as tile
from concourse import bass_utils

N_SRC = 50000
N_DST = 32000
MAX_DEG = 16
N_SUB = 4
D = 128
VOCAB = 32000
N_CLASSES = 104
LN_EPS = 1e-5
T = MAX_DEG - 1
NCORES = 8
P = 128
CH = 512    # free-dim chunk (one PSUM bank of f32)
GCH = 1024  # gather chunk (columns per dma_gather)

F32 = mybir.dt.float32
R32DT = mybir.dt.float32r
BF16 = mybir.dt.bfloat16
I16 = mybir.dt.int16
AF = mybir.ActivationFunctionType
ALU = mybir.AluOpType
BF16_NP = mybir.dt.np(BF16)


def _up128(n):
    return (n + 127) // 128 * 128


# ----------------------------------------------------------------- host prep

def _schedule(deg):
    """Shared per-core schedule from the degree histogram."""
    counts = np.bincount(deg, minlength=MAX_DEG + 1)
    # Pad each degree class to a multiple of 2*NCORES: every core gets the
    # same class counts AND every per-core count is even, so all chunk
    # widths are even (fp32r matmuls require an even moving free dim).
    pm = 2 * NCORES
    cnt_pad = ((counts + pm - 1) // pm) * pm
    cnt_core = cnt_pad // NCORES
    deg_col = np.concatenate(
        [np.full(cnt_core[d], d, np.int64) for d in range(MAX_DEG, 0, -1)]
    )
    Ncol = int(len(deg_col))
    n_t = [int((deg_col >= t + 2).sum()) for t in range(T)]
    N2 = n_t[0]
    Nd1 = Ncol - N2
    kd1 = (Nd1 + P - 1) // P

    # index-buffer offsets (int16 columns), one entry per (step|d1, gchunk)
    offs = {}
    off = 0
    def add(key, w):
        nonlocal off
        L = N_SUB * _up128(w)
        offs[key] = (off, L)
        off += L // 16
    for t in range(T):
        for c0 in range(0, n_t[t], GCH):
            add((t, c0), min(GCH, n_t[t] - c0))
    for c0 in range(0, Nd1, GCH):
        add(("d1", c0), min(GCH, Nd1 - c0))
    return dict(cnt_pad=cnt_pad, Ncol=Ncol, N2=N2, Nd1=Nd1, n_t=n_t, kd1=kd1,
                offs=offs, F16=off)


def _prep(token_id, neigh_idx, deg):
    token_id = np.asarray(token_id).astype(np.int16)  # VOCAB=32000 < 2**15
    neigh_idx = np.asarray(neigh_idx).astype(np.int64)
    deg = np.asarray(deg).astype(np.int64)
    sched = _schedule(deg)
    cnt_pad = sched["cnt_pad"]

    colnode = [[] for _ in range(NCORES)]
    for d in range(MAX_DEG, 0, -1):
        ids = np.where(deg == d)[0].astype(np.int64)
        padded = np.concatenate([ids, np.full(cnt_pad[d] - len(ids), -1, np.int64)])
        for c in range(NCORES):
            colnode[c].append(padded[c::NCORES])
    colnode = [np.concatenate(x) for x in colnode]

    F16 = sched["F16"]
    idxbufs = []
    for c in range(NCORES):
        nid = colnode[c]
        buf = np.zeros((P, F16), np.int16)

        def pack(nida, t_col, w, off, L):
            w32 = L // N_SUB
            src = np.where(nida >= 0, neigh_idx[np.clip(nida, 0, None), t_col], 0)
            tok4 = np.zeros((w32, N_SUB), np.int16)
            tok4[:w] = token_id[src]                      # [w, 4]
            # i = s*w32 + m  ->  tok4[m, s]  (4 contiguous subtoken regions)
            seq = tok4.T.reshape(-1)
            p16 = seq.reshape(L // 16, 16).T              # [16, L/16]
            buf[:, off:off + L // 16] = np.tile(p16, (8, 1))

        for (key, (off, L)) in sched["offs"].items():
            if key[0] == "d1":
                c0 = key[1]
                w = min(GCH, sched["Nd1"] - c0)
                pack(nid[sched["N2"] + c0: sched["N2"] + c0 + w], 0, w, off, L)
            else:
                t, c0 = key
                w = min(GCH, sched["n_t"][t] - c0)
                pack(nid[c0:c0 + w], t, w, off, L)
        idxbufs.append(buf)
    return sched, idxbufs, colnode


def _params(emb, w_ih, w_hh, b_ih, b_hh, gamma, beta, wc, bc):
    f = np.float32
    emb = np.asarray(emb).astype(f)
    w_ih = np.asarray(w_ih).astype(f)
    w_hh = np.asarray(w_hh).astype(f)
    b_ih = np.asarray(b_ih).astype(f)
    b_hh = np.asarray(b_hh).astype(f)
    idm = np.eye(P, dtype=np.float32)
    return dict(
        embbf=np.ascontiguousarray(emb.astype(BF16_NP)),          # [VOCAB, D]
        idbf=np.ascontiguousarray(idm.astype(BF16_NP)),           # [P, P]
        w_ihT_bf=np.ascontiguousarray((0.25 * w_ih.T).astype(BF16_NP)),  # [D,3D]
        w_hhT=np.ascontiguousarray(w_hh.T),          # [D, 3D]
        brc=(b_ih[0:D] + b_hh[0:D]).reshape(D, 1).copy(),
        bzc=(b_ih[D:2 * D] + b_hh[D:2 * D]).reshape(D, 1).copy(),
        bihn=b_ih[2 * D:3 * D].reshape(D, 1).copy(),
        bhhn=b_hh[2 * D:3 * D].reshape(1, D).copy(),
        wcT=np.ascontiguousarray(np.asarray(wc).astype(f).T),   # [D, n_classes]
        bcc=np.asarray(bc).astype(f).reshape(N_CLASSES, 1).copy(),
        gamma=np.asarray(gamma).astype(f).reshape(D, 1).copy(),
        beta=np.asarray(beta).astype(f).reshape(D, 1).copy(),
    )


# ------------------------------------------------------------ device program

def _build_program(sched):
    n_t = sched["n_t"]
    N2, Nd1, kd1, Ncol = sched["N2"], sched["Nd1"], sched["kd1"], sched["Ncol"]
    offs, F16 = sched["offs"], sched["F16"]
    Hw = N2 + kd1 * P  # h tile free size (>= Ncol)
    LMAX = N_SUB * _up128(min(GCH, max(n_t[0], 1)))

    nc = bacc.Bacc("TRN2", target_bir_lowering=False, debug=False,
                   enable_asserts=False, num_swdge_queues=4)

    embbf = nc.dram_tensor("embbf", [VOCAB, D], BF16, kind="ExternalInput").ap()
    idbf_d = nc.dram_tensor("idbf", [P, P], BF16, kind="ExternalInput").ap()
    idxbuf = nc.dram_tensor("idxbuf", [P, F16], I16, kind="ExternalInput").ap()
    w_ihT_bf_d = nc.dram_tensor("w_ihT_bf", [D, 3 * D], BF16, kind="ExternalInput").ap()
    w_hhT_d = nc.dram_tensor("w_hhT", [D, 3 * D], F32, kind="ExternalInput").ap()
    brc_d = nc.dram_tensor("brc", [D, 1], F32, kind="ExternalInput").ap()
    bzc_d = nc.dram_tensor("bzc", [D, 1], F32, kind="ExternalInput").ap()
    bihn_d = nc.dram_tensor("bihn", [D, 1], F32, kind="ExternalInput").ap()
    bhhn_d = nc.dram_tensor("bhhn", [1, D], F32, kind="ExternalInput").ap()
    wcT_d = nc.dram_tensor("wcT", [D, N_CLASSES], F32, kind="ExternalInput").ap()
    bcc_d = nc.dram_tensor("bcc", [N_CLASSES, 1], F32, kind="ExternalInput").ap()
    gamma_d = nc.dram_tensor("gamma", [D, 1], F32, kind="ExternalInput").ap()
    beta_d = nc.dram_tensor("beta", [D, 1], F32, kind="ExternalInput").ap()
    outT = nc.dram_tensor("outT", [N_CLASSES, Ncol], F32, kind="ExternalOutput").ap()

    qctr = [0]
    def next_q():
        q = qctr[0] % 4
        qctr[0] += 1
        return q

    with tile.TileContext(nc) as tc, \
         tc.tile_pool(name="consts", bufs=1) as consts, \
         tc.tile_pool(name="graw", bufs=6) as graw, \
         tc.tile_pool(name="gtmp", bufs=2) as gtmp, \
         tc.tile_pool(name="msum", bufs=4) as msump, \
         tc.tile_pool(name="gpool", bufs=2) as gpool, \
         tc.tile_pool(name="hpool", bufs=1) as hpool, \
         tc.tile_pool(name="opool", bufs=2) as opool, \
         tc.tile_pool(name="ppool", bufs=1, space="PSUM") as ppool, \
         tc.tile_pool(name="trpool", bufs=2, space="PSUM") as trpool:

        def load(name, dram, shape, dtype=F32):
            t = consts.tile(shape, dtype, name=name)
            nc.sync.dma_start(out=t[:], in_=dram)
            return t

        sb_idx = load("sb_idx", idxbuf, [P, F16], I16)
        sbId = load("sbId", idbf_d, [P, P], BF16)
        w_ihT = load("w_ihT_sb", w_ihT_bf_d, [D, 3 * D], BF16)
        w_hhT_f = load("w_hhT_f", w_hhT_d, [D, 3 * D])
        brc = load("brc_sb", brc_d, [D, 1])
        bzc = load("bzc_sb", bzc_d, [D, 1])
        bihn = load("bihn_sb", bihn_d, [D, 1])
        bhhn_f = load("bhhn_f", bhhn_d, [1, D])
        wcT_f = load("wcT_f", wcT_d, [D, N_CLASSES])
        bcc = load("bcc_sb", bcc_d, [N_CLASSES, 1])
        gamma = load("gamma_sb", gamma_d, [D, 1])
        beta = load("beta_sb", beta_d, [D, 1])

        # bf16 copies of everything that feeds bf16 matmuls
        w_hhT = consts.tile([D, 3 * D], BF16, name="w_hhT_bf")
        nc.scalar.copy(w_hhT[:], w_hhT_f[:])
        bhhn = consts.tile([1, D], R32DT, name="bhhn_r")
        nc.scalar.copy(bhhn[:], bhhn_f[:])
        wcT = consts.tile([D, N_CLASSES], BF16, name="wcT_bf")
        nc.scalar.copy(wcT[:], wcT_f[:])

        ones_f = consts.tile([1, CH], F32, name="ones_f")
        nc.vector.memset(ones_f[:], 1.0)
        ones_r = consts.tile([1, CH], R32DT, name="ones_r")
        nc.scalar.copy(ones_r[:], ones_f[:])
        onePf = consts.tile([D, 1], F32, name="onePf")
        nc.vector.memset(onePf[:], 1.0 / D)
        oneP = consts.tile([D, 1], BF16, name="oneP_bf")
        nc.scalar.copy(oneP[:], onePf[:])
        eps_row = consts.tile([1, 1], F32, name="eps_row")
        nc.vector.memset(eps_row[:], LN_EPS)
        quarter = consts.tile([D, 1], F32, name="quarter")
        nc.vector.memset(quarter[:], 0.25)

        h = hpool.tile([P, Hw], BF16, name="h")

        def gather4(key):
            """bf16 gather of 4 subtoken rows per column (4 contiguous
            subtoken regions in the free dim)."""
            off, L = offs[key]
            S4 = graw.tile([P, L], BF16, tag="S4", name="S4",
                           padded_shape=[P, LMAX])
            nc.gpsimd.dma_gather(
                out_ap=S4[:, :L].rearrange("p (j e) -> p j e", e=P),
                in_ap=embbf, idxs_ap=sb_idx[:, off:off + L // 16],
                num_idxs=L, num_idxs_reg=L, elem_size=P,
                single_packet=False, queue_num=next_q())
            return S4

        def sub_sum(S4, key):
            """Subtoken sum: 3 contiguous bf16 DVE adds.  Returns
            [P, up128(w)] bf16 tile in row layout (node q*128+p on
            partition p, 128-elem block q)."""
            off, L = offs[key]
            w128 = L // N_SUB
            a1 = gtmp.tile([P, GCH], BF16, tag="a1", name="a1")
            nc.vector.tensor_add(a1[:, :w128], S4[:, 0:w128],
                                 S4[:, w128:2 * w128])
            a2 = gtmp.tile([P, GCH], BF16, tag="a2", name="a2")
            nc.vector.tensor_add(a2[:, :w128], S4[:, 2 * w128:3 * w128],
                                 S4[:, 3 * w128:4 * w128])
            msr = msump.tile([P, GCH], BF16, tag="msr", name="msr")
            nc.vector.tensor_add(msr[:, :w128], a1[:, :w128], a2[:, :w128])
            return msr

        def trans_sum(msr, e0, w):
            """PE transpose of w node-rows starting at element offset e0 in
            msr.  Returns PSUM tile [P, w]: column m = feat-sum of node m."""
            psumT = trpool.tile([P, CH], BF16, tag="tr", name="psumT")
            for q in range((w + P - 1) // P):
                nc.tensor.transpose(psumT[:, q * P:(q + 1) * P],
                                    msr[:, e0 + q * P: e0 + (q + 1) * P],
                                    sbId[:])
            return psumT

        # ---------------- GRU sweep over steps (descending-degree prefix) --
        # Software-pipelined: gather item k+2 || subtoken-sum item k+1 ||
        # GRU/deg1-consume item k, so the in-order engine queues stay fed.
        items = []
        for t in range(T):
            for g0 in range(0, n_t[t], GCH):
                items.append((t, g0, min(GCH, n_t[t] - g0)))
        for g0 in range(0, Nd1, GCH):
            items.append(("d1", g0, min(GCH, Nd1 - g0)))

        def consume(item, msr):
            t, g0, wg = item
            if t == "d1":
                for c0 in range(g0, g0 + wg, CH):
                    w = min(CH, g0 + wg - c0)
                    psumT = trans_sum(msr, c0 - g0, w)
                    nc.vector.tensor_scalar(
                        out=h[:, N2 + c0: N2 + c0 + w], in0=psumT[:, :w],
                        scalar1=quarter[:], scalar2=None, op0=ALU.mult)
                return
            for c0 in range(g0, g0 + wg, CH):
                    w = min(CH, g0 + wg - c0)
                    cc = slice(c0, c0 + w)
                    lc = slice(0, w)
                    psumT = trans_sum(msr, c0 - g0, w)
                    ms = msump.tile([P, CH], BF16, tag="ms", name="ms")
                    nc.scalar.copy(ms[:, :w], psumT[:, :w])

                    pr = ppool.tile([P, CH], F32, tag="pa", name="pr")
                    pz = ppool.tile([P, CH], F32, tag="pb", name="pz")
                    pnh = ppool.tile([P, CH], F32, tag="pc", name="pnh")
                    pnx = ppool.tile([P, CH], F32, tag="pd", name="pnx")

                    nc.tensor.matmul(pr[:, :w], w_ihT[:, 0:D], ms[:, lc],
                                     start=True, stop=(t == 0))
                    nc.tensor.matmul(pz[:, :w], w_ihT[:, D:2 * D], ms[:, lc],
                                     start=True, stop=(t == 0))
                    nc.tensor.matmul(pnx[:, :w], w_ihT[:, 2 * D:3 * D], ms[:, lc],
                                     start=True, stop=True)
                    nc.tensor.matmul(pnh[:, :w], bhhn[:1, :], ones_r[:1, :w],
                                     start=True, stop=(t == 0))
                    if t > 0:
                        nc.tensor.matmul(pr[:, :w], w_hhT[:, 0:D], h[:, cc],
                                         start=False, stop=True)
                        nc.tensor.matmul(pz[:, :w], w_hhT[:, D:2 * D], h[:, cc],
                                         start=False, stop=True)
                        nc.tensor.matmul(pnh[:, :w], w_hhT[:, 2 * D:3 * D], h[:, cc],
                                         start=False, stop=True)

                    r = gpool.tile([P, CH], BF16, tag="r", name="r")
                    z = gpool.tile([P, CH], BF16, tag="z", name="z")
                    nc.scalar.activation(r[:, :w], pr[:, :w], AF.Sigmoid, bias=brc[:])
                    nc.scalar.activation(z[:, :w], pz[:, :w], AF.Sigmoid, bias=bzc[:])
                    rhn = gpool.tile([P, CH], BF16, tag="rhn", name="rhn")
                    nc.vector.tensor_mul(rhn[:, :w], r[:, :w], pnh[:, :w])
                    t1 = gpool.tile([P, CH], BF16, tag="t1", name="t1")
                    nc.vector.tensor_add(t1[:, :w], rhn[:, :w], pnx[:, :w])
                    nv = gpool.tile([P, CH], BF16, tag="nv", name="nv")
                    nc.scalar.activation(nv[:, :w], t1[:, :w], AF.Tanh, bias=bihn[:])
                    if t == 0:
                        zm = gpool.tile([P, CH], BF16, tag="zm", name="zm")
                        nc.vector.tensor_mul(zm[:, :w], z[:, :w], nv[:, :w])
                        nc.vector.tensor_sub(h[:, cc], nv[:, :w], zm[:, :w])
                    else:
                        hmn = gpool.tile([P, CH], BF16, tag="hmn", name="hmn")
                        nc.vector.tensor_sub(hmn[:, :w], h[:, cc], nv[:, :w])
                        zm = gpool.tile([P, CH], BF16, tag="zm", name="zm")
                        nc.vector.tensor_mul(zm[:, :w], z[:, :w], hmn[:, :w])
                        nc.vector.tensor_add(h[:, cc], zm[:, :w], nv[:, :w])

        def ln_cls(a, b, do_ln):
            """LayerNorm (optional) + classifier for final columns [a, b)."""
            for c0 in range(a, b, CH):
                w = min(CH, b - c0)
                cc = slice(c0, c0 + w)
                if do_ln:
                    pmu = ppool.tile([1, CH], F32, tag="pa", name="pmu")
                    nc.tensor.matmul(pmu[:1, :w], oneP[:], h[:, cc],
                                     start=True, stop=True)
                    sq = gpool.tile([P, CH], BF16, tag="r", name="sq")
                    nc.vector.tensor_mul(sq[:, :w], h[:, cc], h[:, cc])
                    ps2 = ppool.tile([1, CH], F32, tag="pb", name="ps2")
                    nc.tensor.matmul(ps2[:1, :w], oneP[:], sq[:, :w],
                                     start=True, stop=True)
                    mu = gpool.tile([1, CH], F32, tag="mu", name="mu")
                    nc.scalar.copy(mu[:, :w], pmu[:1, :w])
                    m2 = gpool.tile([1, CH], F32, tag="m2", name="m2")
                    nc.vector.tensor_mul(m2[:, :w], mu[:, :w], mu[:, :w])
                    var = gpool.tile([1, CH], F32, tag="var", name="var")
                    nc.vector.tensor_sub(var[:, :w], ps2[:1, :w], m2[:, :w])
                    std = gpool.tile([1, CH], F32, tag="std", name="std")
                    nc.scalar.activation(std[:, :w], var[:, :w], AF.Sqrt,
                                         bias=eps_row[:])
                    rstd = gpool.tile([1, CH], F32, tag="rstd", name="rstd")
                    nc.vector.reciprocal(rstd[:, :w], std[:, :w])
                    nmr = gpool.tile([1, CH], F32, tag="nmr", name="nmr")
                    nc.vector.tensor_mul(nmr[:, :w], mu[:, :w], rstd[:, :w])
                    pa = ppool.tile([P, CH], F32, tag="pc", name="pa_b")
                    nc.tensor.matmul(pa[:, :w], ones_f[:1, :P], rstd[:1, :w],
                                     start=True, stop=True)
                    pb = ppool.tile([P, CH], F32, tag="pd", name="pb_b")
                    nc.tensor.matmul(pb[:, :w], ones_f[:1, :P], nmr[:1, :w],
                                     start=True, stop=True)
                    tl = gpool.tile([P, CH], F32, tag="z", name="tl")
                    nc.vector.tensor_mul(tl[:, :w], h[:, cc], pa[:, :w])
                    t2 = gpool.tile([P, CH], F32, tag="nv", name="t2")
                    nc.vector.tensor_sub(t2[:, :w], tl[:, :w], pb[:, :w])
                    nc.vector.tensor_scalar(
                        out=h[:, cc], in0=t2[:, :w], scalar1=gamma[:],
                        scalar2=beta[:], op0=ALU.mult, op1=ALU.add)
                pcl = ppool.tile([N_CLASSES, CH], F32, tag="pa", name="pcl")
                nc.tensor.matmul(pcl[:N_CLASSES, :w], wcT[:], h[:, cc],
                                 start=True, stop=True)
                ot = opool.tile([N_CLASSES, CH], F32, tag="ot", name="ot")
                nc.scalar.activation(ot[:N_CLASSES, :w], pcl[:N_CLASSES, :w],
                                     AF.Identity, bias=bcc[:])
                nc.sync.dma_start(out=outT[:, cc], in_=ot[:N_CLASSES, :w])

        # ---------------- pipeline driver ---------------------------------
        def key_of(item):
            t, g0, wg = item
            return (t, g0) if t != "d1" else ("d1", g0)

        def finalize(k):
            t, g0, wg = items[k]
            if t == "d1":
                ln_cls(N2 + g0, N2 + g0 + wg, do_ln=False)
                return
            last_of_step = (k + 1 == len(items)) or (items[k + 1][0] != t)
            if not last_of_step:
                return
            if t == T - 1:
                ln_cls(0, n_t[t], do_ln=True)
            elif n_t[t + 1] < n_t[t]:
                ln_cls(n_t[t + 1], n_t[t], do_ln=True)

        raw = {}
        summed = {}
        for i in range(len(items) + 2):
            if i < len(items):
                raw[i] = gather4(key_of(items[i]))
            j = i - 1
            if 0 <= j < len(items):
                summed[j] = sub_sum(raw.pop(j), key_of(items[j]))
            k = i - 2
            if 0 <= k < len(items):
                consume(items[k], summed.pop(k))
                finalize(k)

    nc.compile()
    return nc


_PROGRAM_CACHE = {}


def _program(sched):
    key = (tuple(sched["n_t"]), sched["Ncol"], sched["kd1"])
    if key not in _PROGRAM_CACHE:
        _PROGRAM_CACHE[key] = _build_program(sched)
    return _PROGRAM_CACHE[key]


# ----------------------------------------------------------------- interface

def _in_maps(inputs):
    sched, idxbufs, colnode = _prep(
        inputs["token_id"], inputs["neigh_idx"], inputs["deg"])
    params = _params(
        inputs["emb"], inputs["w_ih"], inputs["w_hh"], inputs["b_ih"],
        inputs["b_hh"], inputs["gamma"], inputs["beta"], inputs["wc"],
        inputs["bc"])
    maps = [dict(params, idxbuf=idxbufs[c]) for c in range(NCORES)]
    return sched, maps, colnode


def _post(results, sched, colnode):
    out = np.zeros((N_DST, N_CLASSES), np.float32)
    for c in range(NCORES):
        oT = np.asarray(results[c]["outT"])  # [n_classes, Ncol]
        nid = colnode[c]
        valid = nid >= 0
        out[nid[valid]] = oT.T[valid]
    return out


def run(inputs, trace=False):
    sched, maps, colnode = _in_maps(inputs)
    nc = _program(sched)
    res = bass_utils.run_bass_kernel_spmd(
        nc, maps, core_ids=list(range(NCORES)), trace=trace)
    return _post(res.results, sched, colnode), res


def kernel(**inputs) -> np.ndarray:
    out, _ = run(inputs, trace=False)
    return out


# ------------------------------------------------- cached-jit timing harness

class TimedRunner:
    """Build the PJRT executable once; re-invoke on device-resident inputs."""

    def __init__(self, inputs):
        import jax
        from jax.sharding import Mesh, PartitionSpec
        from jax.experimental.shard_map import shard_map
        from concourse import bass2jax

        self.sched, maps, self.colnode = _in_maps(inputs)
        nc = _program(self.sched)
        bass2jax.install_neuronx_cc_hook()

        part_name = (nc.partition_id_tensor.name
                     if nc.partition_id_tensor else None)
        in_names, out_names, out_avals, zero_outs = [], [], [], []
        for alloc in nc.m.functions[0].allocations:
            if not isinstance(alloc, mybir.MemoryLocationSet):
                continue
            name = alloc.memorylocations[0].name
            if alloc.kind == "ExternalInput":
                if name != part_name:
                    in_names.append(name)
            elif alloc.kind == "ExternalOutput":
                out_names.append(name)
                dt_np = mybir.dt.np(alloc.dtype)
                out_avals.append(jax.core.ShapedArray(tuple(alloc.tensor_shape), dt_np))
                zero_outs.append(np.zeros(tuple(alloc.tensor_shape), dt_np))
        n_params = len(in_names)
        all_names = in_names + out_names
        if part_name is not None:
            all_names = all_names + [part_name]

        def _body(*args):
            operands = list(args)
            if part_name is not None:
                operands.append(bass2jax.partition_id_tensor())
            outs = bass2jax._bass_exec_p.bind(
                *operands,
                out_avals=tuple(out_avals),
                in_names=tuple(all_names),
                out_names=tuple(out_names),
                lowering_input_output_aliases=(),
                sim_require_finite=True,
                sim_require_nnan=True,
                nc=nc,
            )
            return tuple(outs)

        devices = jax.devices()[:NCORES]
        mesh = Mesh(np.asarray(devices), ("core",))
        nz = len(zero_outs)
        self.fn = jax.jit(
            shard_map(_body, mesh=mesh,
                      in_specs=(PartitionSpec("core"),) * (n_params + nz),
                      out_specs=(PartitionSpec("core"),) * nz,
                      check_rep=False),
            keep_unused=True)
        concat_in = [np.concatenate([np.asarray(m[n]) for m in maps], axis=0)
                     for n in in_names]
        self.dev_in = [jax.device_put(a) for a in concat_in]
        self.zero_outs = zero_outs
        # Device-resident output buffers: the kernel overwrites outT fully
        # every call, so reusing one buffer across calls is safe and avoids
        # re-uploading zeros through the tunnel on every invocation.
        self.dev_zeros = [
            jax.device_put(np.zeros((NCORES * z.shape[0], *z.shape[1:]), z.dtype))
            for z in zero_outs]
        self.out_names = out_names
        self.out_avals = out_avals
        self.jax = jax

    def __call__(self):
        outs = self.fn(*self.dev_in, *self.dev_zeros)
        self.jax.block_until_ready(outs)
        return outs

    def timed(self, iters=5):
        import time
        self()  # warm-up / compile
        times = []
        for _ in range(iters):
            t0 = time.perf_counter()
            self()
            times.append(time.perf_counter() - t0)
        return min(times)

    def result(self):
        outs = self()
        results = []
        for c in range(NCORES):
            d = {}
            for i, n in enumerate(self.out_names):
                full = np.asarray(outs[i])
                d[n] = full.reshape(NCORES, *self.out_avals[i].shape)[c]
            results.append(d)
        return _post(results, self.sched, self.colnode)


# revision 25
# speedup vs baseline: 1.1209x; 1.1209x over previous
# Trainium2 Bass kernel for nn_GTEProgramClassification (GNN message passing).
#
# Math (see problem reference):
#   feat_src = mean_s emb[token_id[:, s]]              [N_src, D]
#   msgs     = feat_src[neigh_idx]                     [N_dst, max_deg, D]
#   h        = GRU scan over msgs[:, :deg-1]           (per-node step count)
#   ft       = deg==1 ? msgs[:,0] : LayerNorm(h)
#   out      = ft @ wc.T + bc                          [N_dst, n_classes]
#
# Strategy (8 cores, data-parallel over dst nodes):
#   * Host sorts dst nodes by degree (descending) and deals them across the
#     8 cores so every core sees the exact same degree profile (classes are
#     padded to a multiple of 16 with fake nodes). One SPMD program.
#   * At GRU step t the active nodes are a shrinking prefix of the sorted
#     columns; retired columns keep their final h in place.
#   * Message features come from bf16 dma_gathers of the four subtoken
#     embedding rows (host passes emb pre-cast to bf16; int16 indices
#     interleaved so one node's 4 rows land on the SAME partition in 4
#     adjacent 128-element slots).  Subtoken sum = 3 strided bf16 DVE adds
#     (cheap, frees the gather buffer fast); the row->column transpose is
#     one PE transpose-mode op per 128 nodes (identity stationary).  The
#     1/4 mean scale is folded into w_ih on the host (deg-1 nodes apply it
#     explicitly).
#   * Gathers round-robin over all 4 SWDGE queues: descriptor generation for
#     queue q runs on Q7 core pair (2q, 2q+1), so four gathers generate
#     descriptors concurrently (~4x the single-queue gather throughput).
#   * Layout B on chip: tiles are [D=128 partitions, nodes free].  LayerNorm
#     reductions over D are K=128 matmuls with a 1/128 ones vector;
#     per-node scalars are partition-broadcast via K=1 matmuls.
#   * GRU h/weights in float32r (1 cycle/row); x-side weights bf16.

import numpy as np
import ml_dtypes

import concourse.bacc as bacc
import concourse.bass as bass
import concourse.mybir as mybir
import concourse.tile as tile
from concourse import bass_utils

N_SRC = 50000
N_DST = 32000
MAX_DEG = 16
N_SUB = 4
D = 128
VOCAB = 32000
N_CLASSES = 104
LN_EPS = 1e-5
T = MAX_DEG - 1
NCORES = 8
P = 128
CH = 512    # free-dim chunk (one PSUM bank of f32)
GCH = 1024  # gather chunk (columns per dma_gather)

F32 = mybir.dt.float32
R32DT = mybir.dt.float32r
BF16 = mybir.dt.bfloat16
I16 = mybir.dt.int16
AF = mybir.ActivationFunctionType
ALU = mybir.AluOpType
BF16_NP = mybir.dt.np(BF16)


def _up128(n):
    return (n + 127) // 128 * 128


# ----------------------------------------------------------------- host prep

def _schedule(deg):
    """Shared per-core schedule from the degree histogram."""
    counts = np.bincount(deg, minlength=MAX_DEG + 1)
    # Pad each degree class to a multiple of 2*NCORES: every core gets the
    # same class counts AND every per-core count is even, so all chunk
    # widths are even (fp32r matmuls require an even moving free dim).
    pm = 2 * NCORES
    cnt_pad = ((counts + pm - 1) // pm) * pm
    cnt_core = cnt_pad // NCORES
    deg_col = np.concatenate(
        [np.full(cnt_core[d], d, np.int64) for d in range(MAX_DEG, 0, -1)]
    )
    Ncol = int(len(deg_col))
    n_t = [int((deg_col >= t + 2).sum()) for t in range(T)]
    N2 = n_t[0]
    Nd1 = Ncol - N2
    kd1 = (Nd1 + P - 1) // P

    # index-buffer offsets (int16 columns), one entry per (step|d1, gchunk)
    offs = {}
    off = 0
    def add(key, w):
        nonlocal off
        L = N_SUB * _up128(w)
        offs[key] = (off, L)
        off += L // 16
    for t in range(T):
        for c0 in range(0, n_t[t], GCH):
            add((t, c0), min(GCH, n_t[t] - c0))
    for c0 in range(0, Nd1, GCH):
        add(("d1", c0), min(GCH, Nd1 - c0))
    return dict(cnt_pad=cnt_pad, Ncol=Ncol, N2=N2, Nd1=Nd1, n_t=n_t, kd1=kd1,
                offs=offs, F16=off)


def _prep(token_id, neigh_idx, deg):
    token_id = np.asarray(token_id).astype(np.int16)  # VOCAB=32000 < 2**15
    neigh_idx = np.asarray(neigh_idx).astype(np.int64)
    deg = np.asarray(deg).astype(np.int64)
    sched = _schedule(deg)
    cnt_pad = sched["cnt_pad"]

    colnode = [[] for _ in range(NCORES)]
    for d in range(MAX_DEG, 0, -1):
        ids = np.where(deg == d)[0].astype(np.int64)
        padded = np.concatenate([ids, np.full(cnt_pad[d] - len(ids), -1, np.int64)])
        for c in range(NCORES):
            colnode[c].append(padded[c::NCORES])
    colnode = [np.concatenate(x) for x in colnode]

    F16 = sched["F16"]
    idxbufs = []
    for c in range(NCORES):
        nid = colnode[c]
        buf = np.zeros((P, F16), np.int16)

        def pack(nida, t_col, w, off, L):
            w32 = L // N_SUB
            src = np.where(nida >= 0, neigh_idx[np.clip(nida, 0, None), t_col], 0)
            tok4 = np.zeros((w32, N_SUB), np.int16)
            tok4[:w] = token_id[src]                      # [w, 4]
            # i = s*w32 + m  ->  tok4[m, s]  (4 contiguous subtoken regions)
            seq = tok4.T.reshape(-1)
            p16 = seq.reshape(L // 16, 16).T              # [16, L/16]
            buf[:, off:off + L // 16] = np.tile(p16, (8, 1))

        for (key, (off, L)) in sched["offs"].items():
            if key[0] == "d1":
                c0 = key[1]
                w = min(GCH, sched["Nd1"] - c0)
                pack(nid[sched["N2"] + c0: sched["N2"] + c0 + w], 0, w, off, L)
            else:
                t, c0 = key
                w = min(GCH, sched["n_t"][t] - c0)
                pack(nid[c0:c0 + w], t, w, off, L)
        idxbufs.append(buf)
    return sched, idxbufs, colnode


def _params(emb, w_ih, w_hh, b_ih, b_hh, gamma, beta, wc, bc):
    f = np.float32
    emb = np.asarray(emb).astype(f)
    w_ih = np.asarray(w_ih).astype(f)
    w_hh = np.asarray(w_hh).astype(f)
    b_ih = np.asarray(b_ih).astype(f)
    b_hh = np.asarray(b_hh).astype(f)
    idm = np.eye(P, dtype=np.float32)
    return dict(
        embbf=np.ascontiguousarray(emb.astype(BF16_NP)),          # [VOCAB, D]
        idbf=np.ascontiguousarray(idm.astype(BF16_NP)),           # [P, P]
        w_ihT_bf=np.ascontiguousarray((0.25 * w_ih.T).astype(BF16_NP)),  # [D,3D]
        w_hhT=np.ascontiguousarray(w_hh.T),          # [D, 3D]
        brc=(b_ih[0:D] + b_hh[0:D]).reshape(D, 1).copy(),
        bzc=(b_ih[D:2 * D] + b_hh[D:2 * D]).reshape(D, 1).copy(),
        bihn=b_ih[2 * D:3 * D].reshape(D, 1).copy(),
        bhhn=b_hh[2 * D:3 * D].reshape(1, D).copy(),
        wcT=np.ascontiguousarray(np.asarray(wc).astype(f).T),   # [D, n_classes]
        bcc=np.asarray(bc).astype(f).reshape(N_CLASSES, 1).copy(),
        gamma=np.asarray(gamma).astype(f).reshape(D, 1).copy(),
        beta=np.asarray(beta).astype(f).reshape(D, 1).copy(),
    )


# ------------------------------------------------------------ device program

def _build_program(sched):
    n_t = sched["n_t"]
    N2, Nd1, kd1, Ncol = sched["N2"], sched["Nd1"], sched["kd1"], sched["Ncol"]
    offs, F16 = sched["offs"], sched["F16"]
    Hw = N2 + kd1 * P  # h tile free size (>= Ncol)
    LMAX = N_SUB * _up128(min(GCH, max(n_t[0], 1)))

    nc = bacc.Bacc("TRN2", target_bir_lowering=False, debug=False,
                   enable_asserts=False, num_swdge_queues=4)

    embbf = nc.dram_tensor("embbf", [VOCAB, D], BF16, kind="ExternalInput").ap()
    idbf_d = nc.dram_tensor("idbf", [P, P], BF16, kind="ExternalInput").ap()
    idxbuf = nc.dram_tensor("idxbuf", [P, F16], I16, kind="ExternalInput").ap()
    w_ihT_bf_d = nc.dram_tensor("w_ihT_bf", [D, 3 * D], BF16, kind="ExternalInput").ap()
    w_hhT_d = nc.dram_tensor("w_hhT", [D, 3 * D], F32, kind="ExternalInput").ap()
    brc_d = nc.dram_tensor("brc", [D, 1], F32, kind="ExternalInput").ap()
    bzc_d = nc.dram_tensor("bzc", [D, 1], F32, kind="ExternalInput").ap()
    bihn_d = nc.dram_tensor("bihn", [D, 1], F32, kind="ExternalInput").ap()
    bhhn_d = nc.dram_tensor("bhhn", [1, D], F32, kind="ExternalInput").ap()
    wcT_d = nc.dram_tensor("wcT", [D, N_CLASSES], F32, kind="ExternalInput").ap()
    bcc_d = nc.dram_tensor("bcc", [N_CLASSES, 1], F32, kind="ExternalInput").ap()
    gamma_d = nc.dram_tensor("gamma", [D, 1], F32, kind="ExternalInput").ap()
    beta_d = nc.dram_tensor("beta", [D, 1], F32, kind="ExternalInput").ap()
    outT = nc.dram_tensor("outT", [N_CLASSES, Ncol], F32, kind="ExternalOutput").ap()

    qctr = [0]
    def next_q():
        q = qctr[0] % 4
        qctr[0] += 1
        return q

    with tile.TileContext(nc) as tc, \
         tc.tile_pool(name="consts", bufs=1) as consts, \
         tc.tile_pool(name="graw", bufs=6) as graw, \
         tc.tile_pool(name="gtmp", bufs=2) as gtmp, \
         tc.tile_pool(name="msum", bufs=4) as msump, \
         tc.tile_pool(name="gpool", bufs=2) as gpool, \
         tc.tile_pool(name="hpool", bufs=1) as hpool, \
         tc.tile_pool(name="opool", bufs=2) as opool, \
         tc.tile_pool(name="ppool", bufs=1, space="PSUM") as ppool, \
         tc.tile_pool(name="trpool", bufs=2, space="PSUM") as trpool:

        def load(name, dram, shape, dtype=F32):
            t = consts.tile(shape, dtype, name=name)
            nc.sync.dma_start(out=t[:], in_=dram)
            return t

        sb_idx = load("sb_idx", idxbuf, [P, F16], I16)
        sbId = load("sbId", idbf_d, [P, P], BF16)
        w_ihT = load("w_ihT_sb", w_ihT_bf_d, [D, 3 * D], BF16)
        w_hhT_f = load("w_hhT_f", w_hhT_d, [D, 3 * D])
        brc = load("brc_sb", brc_d, [D, 1])
        bzc = load("bzc_sb", bzc_d, [D, 1])
        bihn = load("bihn_sb", bihn_d, [D, 1])
        bhhn_f = load("bhhn_f", bhhn_d, [1, D])
        wcT_f = load("wcT_f", wcT_d, [D, N_CLASSES])
        bcc = load("bcc_sb", bcc_d, [N_CLASSES, 1])
        gamma = load("gamma_sb", gamma_d, [D, 1])
        beta = load("beta_sb", beta_d, [D, 1])

        # bf16 copies of everything that feeds bf16 matmuls
        w_hhT = consts.tile([D, 3 * D], BF16, name="w_hhT_bf")
        nc.vector.tensor_copy(w_hhT[:], w_hhT_f[:])
        bhhn = consts.tile([1, D], R32DT, name="bhhn_r")
        nc.vector.tensor_copy(bhhn[:], bhhn_f[:])
        wcT = consts.tile([D, N_CLASSES], BF16, name="wcT_bf")
        nc.vector.tensor_copy(wcT[:], wcT_f[:])

        ones_f = consts.tile([1, CH], F32, name="ones_f")
        nc.vector.memset(ones_f[:], 1.0)
        ones_r = consts.tile([1, CH], R32DT, name="ones_r")
        nc.vector.tensor_copy(ones_r[:], ones_f[:])
        onePf = consts.tile([D, 1], F32, name="onePf")
        nc.vector.memset(onePf[:], 1.0 / D)
        oneP = consts.tile([D, 1], BF16, name="oneP_bf")
        nc.vector.tensor_copy(oneP[:], onePf[:])
        eps_row = consts.tile([1, 1], F32, name="eps_row")
        nc.vector.memset(eps_row[:], LN_EPS)
        quarter = consts.tile([D, 1], F32, name="quarter")
        nc.vector.memset(quarter[:], 0.25)

        h = hpool.tile([P, Hw], BF16, name="h")

        def gather4(key):
            """bf16 gather of 4 subtoken rows per column (4 contiguous
            subtoken regions in the free dim)."""
            off, L = offs[key]
            S4 = graw.tile([P, L], BF16, tag="S4", name="S4",
                           padded_shape=[P, LMAX])
            nc.gpsimd.dma_gather(
                out_ap=S4[:, :L].rearrange("p (j e) -> p j e", e=P),
                in_ap=embbf, idxs_ap=sb_idx[:, off:off + L // 16],
                num_idxs=L, num_idxs_reg=L, elem_size=P,
                single_packet=False, queue_num=next_q())
            return S4

        def sub_sum(S4, key):
            """Subtoken sum: 3 contiguous bf16 DVE adds.  Returns
            [P, up128(w)] bf16 tile in row layout (node q*128+p on
            partition p, 128-elem block q)."""
            off, L = offs[key]
            w128 = L // N_SUB
            a1 = gtmp.tile([P, GCH], BF16, tag="a1", name="a1")
            nc.vector.tensor_add(a1[:, :w128], S4[:, 0:w128],
                                 S4[:, w128:2 * w128])
            a2 = gtmp.tile([P, GCH], BF16, tag="a2", name="a2")
            nc.vector.tensor_add(a2[:, :w128], S4[:, 2 * w128:3 * w128],
                                 S4[:, 3 * w128:4 * w128])
            msr = msump.tile([P, GCH], BF16, tag="msr", name="msr")
            nc.vector.tensor_add(msr[:, :w128], a1[:, :w128], a2[:, :w128])
            return msr

        def trans_sum(msr, e0, w):
            """PE transpose of w node-rows starting at element offset e0 in
            msr.  Returns PSUM tile [P, w]: column m = feat-sum of node m."""
            psumT = trpool.tile([P, CH], BF16, tag="tr", name="psumT")
            for q in range((w + P - 1) // P):
                nc.tensor.transpose(psumT[:, q * P:(q + 1) * P],
                                    msr[:, e0 + q * P: e0 + (q + 1) * P],
                                    sbId[:])
            return psumT

        # ---------------- GRU sweep over steps (descending-degree prefix) --
        # Software-pipelined: gather item k+2 || subtoken-sum item k+1 ||
        # GRU/deg1-consume item k, so the in-order engine queues stay fed.
        items = []
        for t in range(T):
            for g0 in range(0, n_t[t], GCH):
                items.append((t, g0, min(GCH, n_t[t] - g0)))
        for g0 in range(0, Nd1, GCH):
            items.append(("d1", g0, min(GCH, Nd1 - g0)))

        def consume(item, msr):
            t, g0, wg = item
            if t == "d1":
                for c0 in range(g0, g0 + wg, CH):
                    w = min(CH, g0 + wg - c0)
                    psumT = trans_sum(msr, c0 - g0, w)
                    nc.vector.tensor_scalar(
                        out=h[:, N2 + c0: N2 + c0 + w], in0=psumT[:, :w],
                        scalar1=quarter[:], scalar2=None, op0=ALU.mult)
                return
            for c0 in range(g0, g0 + wg, CH):
                    w = min(CH, g0 + wg - c0)
                    cc = slice(c0, c0 + w)
                    lc = slice(0, w)
                    psumT = trans_sum(msr, c0 - g0, w)
                    ms = msump.tile([P, CH], BF16, tag="ms", name="ms")
                    nc.scalar.copy(ms[:, :w], psumT[:, :w])

                    pr = ppool.tile([P, CH], F32, tag="pa", name="pr")
                    pz = ppool.tile([P, CH], F32, tag="pb", name="pz")
                    pnh = ppool.tile([P, CH], F32, tag="pc", name="pnh")
                    pnx = ppool.tile([P, CH], F32, tag="pd", name="pnx")

                    nc.tensor.matmul(pr[:, :w], w_ihT[:, 0:D], ms[:, lc],
                                     start=True, stop=(t == 0))
                    nc.tensor.matmul(pz[:, :w], w_ihT[:, D:2 * D], ms[:, lc],
                                     start=True, stop=(t == 0))
                    nc.tensor.matmul(pnx[:, :w], w_ihT[:, 2 * D:3 * D], ms[:, lc],
                                     start=True, stop=True)
                    nc.tensor.matmul(pnh[:, :w], bhhn[:1, :], ones_r[:1, :w],
                                     start=True, stop=(t == 0))
                    if t > 0:
                        nc.tensor.matmul(pr[:, :w], w_hhT[:, 0:D], h[:, cc],
                                         start=False, stop=True)
                        nc.tensor.matmul(pz[:, :w], w_hhT[:, D:2 * D], h[:, cc],
                                         start=False, stop=True)
                        nc.tensor.matmul(pnh[:, :w], w_hhT[:, 2 * D:3 * D], h[:, cc],
                                         start=False, stop=True)

                    r = gpool.tile([P, CH], BF16, tag="r", name="r")
                    z = gpool.tile([P, CH], BF16, tag="z", name="z")
                    nc.scalar.activation(r[:, :w], pr[:, :w], AF.Sigmoid, bias=brc[:])
                    nc.scalar.activation(z[:, :w], pz[:, :w], AF.Sigmoid, bias=bzc[:])
                    rhn = gpool.tile([P, CH], BF16, tag="rhn", name="rhn")
                    nc.vector.tensor_mul(rhn[:, :w], r[:, :w], pnh[:, :w])
                    t1 = gpool.tile([P, CH], BF16, tag="t1", name="t1")
                    nc.vector.tensor_add(t1[:, :w], rhn[:, :w], pnx[:, :w])
                    nv = gpool.tile([P, CH], BF16, tag="nv", name="nv")
                    nc.scalar.activation(nv[:, :w], t1[:, :w], AF.Tanh, bias=bihn[:])
                    if t == 0:
                        zm = gpool.tile([P, CH], BF16, tag="zm", name="zm")
                        nc.vector.tensor_mul(zm[:, :w], z[:, :w], nv[:, :w])
                        nc.vector.tensor_sub(h[:, cc], nv[:, :w], zm[:, :w])
                    else:
                        hmn = gpool.tile([P, CH], BF16, tag="hmn", name="hmn")
                        nc.vector.tensor_sub(hmn[:, :w], h[:, cc], nv[:, :w])
                        zm = gpool.tile([P, CH], BF16, tag="zm", name="zm")
                        nc.vector.tensor_mul(zm[:, :w], z[:, :w], hmn[:, :w])
                        nc.vector.tensor_add(h[:, cc], zm[:, :w], nv[:, :w])

        # ---------------- pipeline driver ---------------------------------
        def key_of(item):
            t, g0, wg = item
            return (t, g0) if t != "d1" else ("d1", g0)

        raw = {}
        summed = {}
        for i in range(len(items) + 2):
            if i < len(items):
                raw[i] = gather4(key_of(items[i]))
            j = i - 1
            if 0 <= j < len(items):
                summed[j] = sub_sum(raw.pop(j), key_of(items[j]))
            k = i - 2
            if 0 <= k < len(items):
                consume(items[k], summed.pop(k))

        # ---------------- LayerNorm over D (partitions) for cols [0, N2) --
        for c0 in range(0, N2, CH):
            w = min(CH, N2 - c0)
            cc = slice(c0, c0 + w)
            pmu = ppool.tile([1, CH], F32, tag="pa", name="pmu")
            nc.tensor.matmul(pmu[:1, :w], oneP[:], h[:, cc], start=True, stop=True)
            sq = gpool.tile([P, CH], BF16, tag="r", name="sq")
            nc.vector.tensor_mul(sq[:, :w], h[:, cc], h[:, cc])
            ps2 = ppool.tile([1, CH], F32, tag="pb", name="ps2")
            nc.tensor.matmul(ps2[:1, :w], oneP[:], sq[:, :w], start=True, stop=True)
            mu = gpool.tile([1, CH], F32, tag="mu", name="mu")
            nc.scalar.copy(mu[:, :w], pmu[:1, :w])
            m2 = gpool.tile([1, CH], F32, tag="m2", name="m2")
            nc.vector.tensor_mul(m2[:, :w], mu[:, :w], mu[:, :w])
            var = gpool.tile([1, CH], F32, tag="var", name="var")
            nc.vector.tensor_sub(var[:, :w], ps2[:1, :w], m2[:, :w])
            std = gpool.tile([1, CH], F32, tag="std", name="std")
            nc.scalar.activation(std[:, :w], var[:, :w], AF.Sqrt, bias=eps_row[:])
            rstd = gpool.tile([1, CH], F32, tag="rstd", name="rstd")
            nc.vector.reciprocal(rstd[:, :w], std[:, :w])
            nmr = gpool.tile([1, CH], F32, tag="nmr", name="nmr")
            nc.vector.tensor_mul(nmr[:, :w], mu[:, :w], rstd[:, :w])
            pa = ppool.tile([P, CH], F32, tag="pc", name="pa_b")
            nc.tensor.matmul(pa[:, :w], ones_f[:1, :P], rstd[:1, :w],
                             start=True, stop=True)
            pb = ppool.tile([P, CH], F32, tag="pd", name="pb_b")
            nc.tensor.matmul(pb[:, :w], ones_f[:1, :P], nmr[:1, :w],
                             start=True, stop=True)
            tl = gpool.tile([P, CH], F32, tag="z", name="tl")
            nc.vector.tensor_mul(tl[:, :w], h[:, cc], pa[:, :w])
            t2 = gpool.tile([P, CH], F32, tag="nv", name="t2")
            nc.vector.tensor_sub(t2[:, :w], tl[:, :w], pb[:, :w])
            nc.vector.tensor_scalar(
                out=h[:, cc], in0=t2[:, :w], scalar1=gamma[:], scalar2=beta[:],
                op0=ALU.mult, op1=ALU.add)

        # ---------------- classifier: outT = wc @ ft + bc ------------------
        for c0 in range(0, Ncol, CH):
            w = min(CH, Ncol - c0)
            cc = slice(c0, c0 + w)
            pcl = ppool.tile([N_CLASSES, CH], F32, tag="pa", name="pcl")
            nc.tensor.matmul(pcl[:N_CLASSES, :w], wcT[:], h[:, cc],
                             start=True, stop=True)
            ot = opool.tile([N_CLASSES, CH], F32, tag="ot", name="ot")
            nc.scalar.activation(ot[:N_CLASSES, :w], pcl[:N_CLASSES, :w],
                                 AF.Identity, bias=bcc[:])
            nc.sync.dma_start(out=outT[:, cc], in_=ot[:N_CLASSES, :w])

    nc.compile()
    return nc


_PROGRAM_CACHE = {}


def _program(sched):
    key = (tuple(sched["n_t"]), sched["Ncol"], sched["kd1"])
    if key not in _PROGRAM_CACHE:
        _PROGRAM_CACHE[key] = _build_program(sched)
    return _PROGRAM_CACHE[key]


# ----------------------------------------------------------------- interface

def _in_maps(inputs):
    sched, idxbufs, colnode = _prep(
        inputs["token_id"], inputs["neigh_idx"], inputs["deg"])
    params = _params(
        inputs["emb"], inputs["w_ih"], inputs["w_hh"], inputs["b_ih"],
        inputs["b_hh"], inputs["gamma"], inputs["beta"], inputs["wc"],
        inputs["bc"])
    maps = [dict(params, idxbuf=idxbufs[c]) for c in range(NCORES)]
    return sched, maps, colnode


def _post(results, sched, colnode):
    out = np.zeros((N_DST, N_CLASSES), np.float32)
    for c in range(NCORES):
        oT = np.asarray(results[c]["outT"])  # [n_classes, Ncol]
        nid = colnode[c]
        valid = nid >= 0
        out[nid[valid]] = oT.T[valid]
    return out


def run(inputs, trace=False):
    sched, maps, colnode = _in_maps(inputs)
    nc = _program(sched)
    res = bass_utils.run_bass_kernel_spmd(
        nc, maps, core_ids=list(range(NCORES)), trace=trace)
    return _post(res.results, sched, colnode), res


def kernel(**inputs) -> np.ndarray:
    out, _ = run(inputs, trace=False)
    return out


# ------------------------------------------------- cached-jit timing harness

class TimedRunner:
    """Build the PJRT executable once; re-invoke on device-resident inputs."""

    def __init__(self, inputs):
        import jax
        from jax.sharding import Mesh, PartitionSpec
        from jax.experimental.shard_map import shard_map
        from concourse import bass2jax

        self.sched, maps, self.colnode = _in_maps(inputs)
        nc = _program(self.sched)
        bass2jax.install_neuronx_cc_hook()

        part_name = (nc.partition_id_tensor.name
                     if nc.partition_id_tensor else None)
        in_names, out_names, out_avals, zero_outs = [], [], [], []
        for alloc in nc.m.functions[0].allocations:
            if not isinstance(alloc, mybir.MemoryLocationSet):
                continue
            name = alloc.memorylocations[0].name
            if alloc.kind == "ExternalInput":
                if name != part_name:
                    in_names.append(name)
            elif alloc.kind == "ExternalOutput":
                out_names.append(name)
                dt_np = mybir.dt.np(alloc.dtype)
                out_avals.append(jax.core.ShapedArray(tuple(alloc.tensor_shape), dt_np))
                zero_outs.append(np.zeros(tuple(alloc.tensor_shape), dt_np))
        n_params = len(in_names)
        all_names = in_names + out_names
        if part_name is not None:
            all_names = all_names + [part_name]

        def _body(*args):
            operands = list(args)
            if part_name is not None:
                operands.append(bass2jax.partition_id_tensor())
            outs = bass2jax._bass_exec_p.bind(
                *operands,
                out_avals=tuple(out_avals),
                in_names=tuple(all_names),
                out_names=tuple(out_names),
                lowering_input_output_aliases=(),
                sim_require_finite=True,
                sim_require_nnan=True,
                nc=nc,
            )
            return tuple(outs)

        devices = jax.devices()[:NCORES]
        mesh = Mesh(np.asarray(devices), ("core",))
        nz = len(zero_outs)
        self.fn = jax.jit(
            shard_map(_body, mesh=mesh,
                      in_specs=(PartitionSpec("core"),) * (n_params + nz),
                      out_specs=(PartitionSpec("core"),) * nz,
                      check_rep=False),
            keep_unused=True)
        concat_in = [np.concatenate([np.asarray(m[n]) for m in maps], axis=0)
                     for n in in_names]
        self.dev_in = [jax.device_put(a) for a in concat_in]
        self.zero_outs = zero_outs
        # Device-resident output buffers: the kernel overwrites outT fully
        # every call, so reusing one buffer across calls is safe and avoids
        # re-uploading zeros through the tunnel on every invocation.
        self.dev_zeros = [
            jax.device_put(np.zeros((NCORES * z.shape[0], *z.shape[1:]), z.dtype))
            for z in zero_outs]
        self.out_names = out_names
        self.out_avals = out_avals
        self.jax = jax

    def __call__(self):
        outs = self.fn(*self.dev_in, *self.dev_zeros)
        self.jax.block_until_ready(outs)
        return outs

    def timed(self, iters=5):
        import time
        self()  # warm-up / compile
        times = []
        for _ in range(iters):
            t0 = time.perf_counter()
            self()
            times.append(time.perf_counter() - t0)
        return min(times)

    def result(self):
        outs = self()
        results = []
        for c in range(NCORES):
            d = {}
            for i, n in enumerate(self.out_names):
                full = np.asarray(outs[i])
                d[n] = full.reshape(NCORES, *self.out_avals[i].shape)[c]
            results.append(d)
        return _post(results, self.sched, self.colnode)


# revision 26
# speedup vs baseline: 1.5633x; 1.3946x over previous
# Trainium2 Bass kernel for nn_GTEProgramClassification (GNN message passing).
#
# Math (see problem reference):
#   feat_src = mean_s emb[token_id[:, s]]              [N_src, D]
#   msgs     = feat_src[neigh_idx]                     [N_dst, max_deg, D]
#   h        = GRU scan over msgs[:, :deg-1]           (per-node step count)
#   ft       = deg==1 ? msgs[:,0] : LayerNorm(h)
#   out      = ft @ wc.T + bc                          [N_dst, n_classes]
#
# Strategy (8 cores, data-parallel over dst nodes):
#   * Host sorts dst nodes by degree (descending) and deals them across the
#     8 cores so every core sees the exact same degree profile (classes are
#     padded to a multiple of 16 with fake nodes). One SPMD program.
#   * At GRU step t the active nodes are a shrinking prefix of the sorted
#     columns; retired columns keep their final h in place.
#   * Message features come from bf16 dma_gathers of the four subtoken
#     embedding rows (host passes emb pre-cast to bf16; int16 indices
#     interleaved so one node's 4 rows land on the SAME partition in 4
#     adjacent 128-element slots).  Subtoken sum = 3 strided bf16 DVE adds
#     (cheap, frees the gather buffer fast); the row->column transpose is
#     one PE transpose-mode op per 128 nodes (identity stationary).  The
#     1/4 mean scale is folded into w_ih on the host (deg-1 nodes apply it
#     explicitly).
#   * Gathers round-robin over all 4 SWDGE queues: descriptor generation for
#     queue q runs on Q7 core pair (2q, 2q+1), so four gathers generate
#     descriptors concurrently (~4x the single-queue gather throughput).
#   * Layout B on chip: tiles are [D=128 partitions, nodes free].  LayerNorm
#     reductions over D are K=128 matmuls with a 1/128 ones vector;
#     per-node scalars are partition-broadcast via K=1 matmuls.
#   * GRU h/weights in float32r (1 cycle/row); x-side weights bf16.

import numpy as np
import ml_dtypes

import concourse.bacc as bacc
import concourse.bass as bass
import concourse.mybir as mybir
import concourse.tile as tile
from concourse import bass_utils

N_SRC = 50000
N_DST = 32000
MAX_DEG = 16
N_SUB = 4
D = 128
VOCAB = 32000
N_CLASSES = 104
LN_EPS = 1e-5
T = MAX_DEG - 1
NCORES = 8
P = 128
CH = 512    # free-dim chunk (one PSUM bank of f32)
GCH = 512   # gather chunk (columns per dma_gather)

F32 = mybir.dt.float32
R32DT = mybir.dt.float32r
BF16 = mybir.dt.bfloat16
I16 = mybir.dt.int16
AF = mybir.ActivationFunctionType
ALU = mybir.AluOpType
BF16_NP = mybir.dt.np(BF16)


def _up128(n):
    return (n + 127) // 128 * 128


# ----------------------------------------------------------------- host prep

def _schedule(deg):
    """Shared per-core schedule from the degree histogram."""
    counts = np.bincount(deg, minlength=MAX_DEG + 1)
    # Pad each degree class to a multiple of 2*NCORES: every core gets the
    # same class counts AND every per-core count is even, so all chunk
    # widths are even (fp32r matmuls require an even moving free dim).
    pm = 2 * NCORES
    cnt_pad = ((counts + pm - 1) // pm) * pm
    cnt_core = cnt_pad // NCORES
    deg_col = np.concatenate(
        [np.full(cnt_core[d], d, np.int64) for d in range(MAX_DEG, 0, -1)]
    )
    Ncol = int(len(deg_col))
    n_t = [int((deg_col >= t + 2).sum()) for t in range(T)]
    N2 = n_t[0]
    Nd1 = Ncol - N2
    kd1 = (Nd1 + P - 1) // P

    # index-buffer offsets (int16 columns), one entry per (step|d1, gchunk)
    offs = {}
    off = 0
    def add(key, w):
        nonlocal off
        L = N_SUB * _up128(w)
        offs[key] = (off, L)
        off += L // 16
    for t in range(T):
        for c0 in range(0, n_t[t], GCH):
            add((t, c0), min(GCH, n_t[t] - c0))
    for c0 in range(0, Nd1, GCH):
        add(("d1", c0), min(GCH, Nd1 - c0))
    return dict(cnt_pad=cnt_pad, Ncol=Ncol, N2=N2, Nd1=Nd1, n_t=n_t, kd1=kd1,
                offs=offs, F16=off)


def _prep(token_id, neigh_idx, deg):
    token_id = np.asarray(token_id).astype(np.int16)  # VOCAB=32000 < 2**15
    neigh_idx = np.asarray(neigh_idx).astype(np.int64)
    deg = np.asarray(deg).astype(np.int64)
    sched = _schedule(deg)
    cnt_pad = sched["cnt_pad"]

    colnode = [[] for _ in range(NCORES)]
    for d in range(MAX_DEG, 0, -1):
        ids = np.where(deg == d)[0].astype(np.int64)
        padded = np.concatenate([ids, np.full(cnt_pad[d] - len(ids), -1, np.int64)])
        for c in range(NCORES):
            colnode[c].append(padded[c::NCORES])
    colnode = [np.concatenate(x) for x in colnode]

    F16 = sched["F16"]
    idxbufs = []
    for c in range(NCORES):
        nid = colnode[c]
        buf = np.zeros((P, F16), np.int16)

        def pack(nida, t_col, w, off, L):
            w32 = L // N_SUB
            src = np.where(nida >= 0, neigh_idx[np.clip(nida, 0, None), t_col], 0)
            tok4 = np.zeros((w32, N_SUB), np.int16)
            tok4[:w] = token_id[src]                      # [w, 4]
            # i = s*w32 + m  ->  tok4[m, s]  (4 contiguous subtoken regions)
            seq = tok4.T.reshape(-1)
            p16 = seq.reshape(L // 16, 16).T              # [16, L/16]
            buf[:, off:off + L // 16] = np.tile(p16, (8, 1))

        for (key, (off, L)) in sched["offs"].items():
            if key[0] == "d1":
                c0 = key[1]
                w = min(GCH, sched["Nd1"] - c0)
                pack(nid[sched["N2"] + c0: sched["N2"] + c0 + w], 0, w, off, L)
            else:
                t, c0 = key
                w = min(GCH, sched["n_t"][t] - c0)
                pack(nid[c0:c0 + w], t, w, off, L)
        idxbufs.append(buf)
    return sched, idxbufs, colnode


def _params(emb, w_ih, w_hh, b_ih, b_hh, gamma, beta, wc, bc):
    f = np.float32
    emb = np.asarray(emb).astype(f)
    w_ih = np.asarray(w_ih).astype(f)
    w_hh = np.asarray(w_hh).astype(f)
    b_ih = np.asarray(b_ih).astype(f)
    b_hh = np.asarray(b_hh).astype(f)
    idm = np.eye(P, dtype=np.float32)
    return dict(
        embbf=np.ascontiguousarray(emb.astype(BF16_NP)),          # [VOCAB, D]
        idbf=np.ascontiguousarray(idm.astype(BF16_NP)),           # [P, P]
        w_ihT_bf=np.ascontiguousarray((0.25 * w_ih.T).astype(BF16_NP)),  # [D,3D]
        w_hhT=np.ascontiguousarray(w_hh.T),          # [D, 3D]
        brc=(b_ih[0:D] + b_hh[0:D]).reshape(D, 1).copy(),
        bzc=(b_ih[D:2 * D] + b_hh[D:2 * D]).reshape(D, 1).copy(),
        bihn=b_ih[2 * D:3 * D].reshape(D, 1).copy(),
        bhhn=b_hh[2 * D:3 * D].reshape(1, D).copy(),
        wcT=np.ascontiguousarray(np.asarray(wc).astype(f).T),   # [D, n_classes]
        bcc=np.asarray(bc).astype(f).reshape(N_CLASSES, 1).copy(),
        gamma=np.asarray(gamma).astype(f).reshape(D, 1).copy(),
        beta=np.asarray(beta).astype(f).reshape(D, 1).copy(),
    )


# ------------------------------------------------------------ device program

def _build_program(sched):
    n_t = sched["n_t"]
    N2, Nd1, kd1, Ncol = sched["N2"], sched["Nd1"], sched["kd1"], sched["Ncol"]
    offs, F16 = sched["offs"], sched["F16"]
    Hw = N2 + kd1 * P  # h tile free size (>= Ncol)
    LMAX = N_SUB * _up128(min(GCH, max(n_t[0], 1)))

    nc = bacc.Bacc("TRN2", target_bir_lowering=False, debug=False,
                   enable_asserts=False, num_swdge_queues=4)

    embbf = nc.dram_tensor("embbf", [VOCAB, D], BF16, kind="ExternalInput").ap()
    idbf_d = nc.dram_tensor("idbf", [P, P], BF16, kind="ExternalInput").ap()
    idxbuf = nc.dram_tensor("idxbuf", [P, F16], I16, kind="ExternalInput").ap()
    w_ihT_bf_d = nc.dram_tensor("w_ihT_bf", [D, 3 * D], BF16, kind="ExternalInput").ap()
    w_hhT_d = nc.dram_tensor("w_hhT", [D, 3 * D], F32, kind="ExternalInput").ap()
    brc_d = nc.dram_tensor("brc", [D, 1], F32, kind="ExternalInput").ap()
    bzc_d = nc.dram_tensor("bzc", [D, 1], F32, kind="ExternalInput").ap()
    bihn_d = nc.dram_tensor("bihn", [D, 1], F32, kind="ExternalInput").ap()
    bhhn_d = nc.dram_tensor("bhhn", [1, D], F32, kind="ExternalInput").ap()
    wcT_d = nc.dram_tensor("wcT", [D, N_CLASSES], F32, kind="ExternalInput").ap()
    bcc_d = nc.dram_tensor("bcc", [N_CLASSES, 1], F32, kind="ExternalInput").ap()
    gamma_d = nc.dram_tensor("gamma", [D, 1], F32, kind="ExternalInput").ap()
    beta_d = nc.dram_tensor("beta", [D, 1], F32, kind="ExternalInput").ap()
    outT = nc.dram_tensor("outT", [N_CLASSES, Ncol], F32, kind="ExternalOutput").ap()

    qctr = [0]
    def next_q():
        q = qctr[0] % 4
        qctr[0] += 1
        return q

    with tile.TileContext(nc) as tc, \
         tc.tile_pool(name="consts", bufs=1) as consts, \
         tc.tile_pool(name="graw", bufs=8) as graw, \
         tc.tile_pool(name="gtmp", bufs=3) as gtmp, \
         tc.tile_pool(name="msum", bufs=6) as msump, \
         tc.tile_pool(name="gpool", bufs=2) as gpool, \
         tc.tile_pool(name="hpool", bufs=1) as hpool, \
         tc.tile_pool(name="opool", bufs=2) as opool, \
         tc.tile_pool(name="ppool", bufs=1, space="PSUM") as ppool, \
         tc.tile_pool(name="trpool", bufs=2, space="PSUM") as trpool:

        def load(name, dram, shape, dtype=F32):
            t = consts.tile(shape, dtype, name=name)
            nc.sync.dma_start(out=t[:], in_=dram)
            return t

        sb_idx = load("sb_idx", idxbuf, [P, F16], I16)
        sbId = load("sbId", idbf_d, [P, P], BF16)
        w_ihT = load("w_ihT_sb", w_ihT_bf_d, [D, 3 * D], BF16)
        w_hhT_f = load("w_hhT_f", w_hhT_d, [D, 3 * D])
        brc = load("brc_sb", brc_d, [D, 1])
        bzc = load("bzc_sb", bzc_d, [D, 1])
        bihn = load("bihn_sb", bihn_d, [D, 1])
        bhhn_f = load("bhhn_f", bhhn_d, [1, D])
        wcT_f = load("wcT_f", wcT_d, [D, N_CLASSES])
        bcc = load("bcc_sb", bcc_d, [N_CLASSES, 1])
        gamma = load("gamma_sb", gamma_d, [D, 1])
        beta = load("beta_sb", beta_d, [D, 1])

        # bf16 copies of everything that feeds bf16 matmuls
        w_hhT = consts.tile([D, 3 * D], BF16, name="w_hhT_bf")
        nc.vector.tensor_copy(w_hhT[:], w_hhT_f[:])
        bhhn = consts.tile([1, D], R32DT, name="bhhn_r")
        nc.vector.tensor_copy(bhhn[:], bhhn_f[:])
        wcT = consts.tile([D, N_CLASSES], BF16, name="wcT_bf")
        nc.vector.tensor_copy(wcT[:], wcT_f[:])

        ones_f = consts.tile([1, CH], F32, name="ones_f")
        nc.vector.memset(ones_f[:], 1.0)
        ones_r = consts.tile([1, CH], R32DT, name="ones_r")
        nc.vector.tensor_copy(ones_r[:], ones_f[:])
        onePf = consts.tile([D, 1], F32, name="onePf")
        nc.vector.memset(onePf[:], 1.0 / D)
        oneP = consts.tile([D, 1], BF16, name="oneP_bf")
        nc.vector.tensor_copy(oneP[:], onePf[:])
        eps_row = consts.tile([1, 1], F32, name="eps_row")
        nc.vector.memset(eps_row[:], LN_EPS)
        quarter = consts.tile([D, 1], F32, name="quarter")
        nc.vector.memset(quarter[:], 0.25)

        h = hpool.tile([P, Hw], BF16, name="h")

        def gather4(key):
            """bf16 gather of 4 subtoken rows per column (4 contiguous
            subtoken regions in the free dim)."""
            off, L = offs[key]
            S4 = graw.tile([P, L], BF16, tag="S4", name="S4",
                           padded_shape=[P, LMAX])
            nc.gpsimd.dma_gather(
                out_ap=S4[:, :L].rearrange("p (j e) -> p j e", e=P),
                in_ap=embbf, idxs_ap=sb_idx[:, off:off + L // 16],
                num_idxs=L, num_idxs_reg=L, elem_size=P,
                single_packet=False, queue_num=next_q())
            return S4

        def sub_sum(S4, key):
            """Subtoken sum: 3 contiguous bf16 DVE adds.  Returns
            [P, up128(w)] bf16 tile in row layout (node q*128+p on
            partition p, 128-elem block q)."""
            off, L = offs[key]
            w128 = L // N_SUB
            a1 = gtmp.tile([P, GCH], BF16, tag="a1", name="a1")
            nc.vector.tensor_add(a1[:, :w128], S4[:, 0:w128],
                                 S4[:, w128:2 * w128])
            a2 = gtmp.tile([P, GCH], BF16, tag="a2", name="a2")
            nc.vector.tensor_add(a2[:, :w128], S4[:, 2 * w128:3 * w128],
                                 S4[:, 3 * w128:4 * w128])
            msr = msump.tile([P, GCH], BF16, tag="msr", name="msr")
            nc.vector.tensor_add(msr[:, :w128], a1[:, :w128], a2[:, :w128])
            return msr

        def trans_sum(msr, e0, w):
            """PE transpose of w node-rows starting at element offset e0 in
            msr.  Returns PSUM tile [P, w]: column m = feat-sum of node m."""
            psumT = trpool.tile([P, CH], BF16, tag="tr", name="psumT")
            for q in range((w + P - 1) // P):
                nc.tensor.transpose(psumT[:, q * P:(q + 1) * P],
                                    msr[:, e0 + q * P: e0 + (q + 1) * P],
                                    sbId[:])
            return psumT

        # ---------------- GRU sweep over steps (descending-degree prefix) --
        # Software-pipelined: gather item k+2 || subtoken-sum item k+1 ||
        # GRU/deg1-consume item k, so the in-order engine queues stay fed.
        items = []
        for t in range(T):
            for g0 in range(0, n_t[t], GCH):
                items.append((t, g0, min(GCH, n_t[t] - g0)))
        for g0 in range(0, Nd1, GCH):
            items.append(("d1", g0, min(GCH, Nd1 - g0)))

        def consume(item, msr):
            t, g0, wg = item
            if t == "d1":
                for c0 in range(g0, g0 + wg, CH):
                    w = min(CH, g0 + wg - c0)
                    psumT = trans_sum(msr, c0 - g0, w)
                    nc.vector.tensor_scalar(
                        out=h[:, N2 + c0: N2 + c0 + w], in0=psumT[:, :w],
                        scalar1=quarter[:], scalar2=None, op0=ALU.mult)
                return
            for c0 in range(g0, g0 + wg, CH):
                    w = min(CH, g0 + wg - c0)
                    cc = slice(c0, c0 + w)
                    lc = slice(0, w)
                    psumT = trans_sum(msr, c0 - g0, w)
                    ms = msump.tile([P, CH], BF16, tag="ms", name="ms")
                    nc.scalar.copy(ms[:, :w], psumT[:, :w])

                    pr = ppool.tile([P, CH], F32, tag="pa", name="pr")
                    pz = ppool.tile([P, CH], F32, tag="pb", name="pz")
                    pnh = ppool.tile([P, CH], F32, tag="pc", name="pnh")
                    pnx = ppool.tile([P, CH], F32, tag="pd", name="pnx")

                    nc.tensor.matmul(pr[:, :w], w_ihT[:, 0:D], ms[:, lc],
                                     start=True, stop=(t == 0))
                    nc.tensor.matmul(pz[:, :w], w_ihT[:, D:2 * D], ms[:, lc],
                                     start=True, stop=(t == 0))
                    nc.tensor.matmul(pnx[:, :w], w_ihT[:, 2 * D:3 * D], ms[:, lc],
                                     start=True, stop=True)
                    nc.tensor.matmul(pnh[:, :w], bhhn[:1, :], ones_r[:1, :w],
                                     start=True, stop=(t == 0))
                    if t > 0:
                        nc.tensor.matmul(pr[:, :w], w_hhT[:, 0:D], h[:, cc],
                                         start=False, stop=True)
                        nc.tensor.matmul(pz[:, :w], w_hhT[:, D:2 * D], h[:, cc],
                                         start=False, stop=True)
                        nc.tensor.matmul(pnh[:, :w], w_hhT[:, 2 * D:3 * D], h[:, cc],
                                         start=False, stop=True)

                    r = gpool.tile([P, CH], BF16, tag="r", name="r")
                    z = gpool.tile([P, CH], BF16, tag="z", name="z")
                    nc.scalar.activation(r[:, :w], pr[:, :w], AF.Sigmoid, bias=brc[:])
                    nc.scalar.activation(z[:, :w], pz[:, :w], AF.Sigmoid, bias=bzc[:])
                    rhn = gpool.tile([P, CH], BF16, tag="rhn", name="rhn")
                    nc.vector.tensor_mul(rhn[:, :w], r[:, :w], pnh[:, :w])
                    t1 = gpool.tile([P, CH], BF16, tag="t1", name="t1")
                    nc.vector.tensor_add(t1[:, :w], rhn[:, :w], pnx[:, :w])
                    nv = gpool.tile([P, CH], BF16, tag="nv", name="nv")
                    nc.scalar.activation(nv[:, :w], t1[:, :w], AF.Tanh, bias=bihn[:])
                    if t == 0:
                        zm = gpool.tile([P, CH], BF16, tag="zm", name="zm")
                        nc.vector.tensor_mul(zm[:, :w], z[:, :w], nv[:, :w])
                        nc.vector.tensor_sub(h[:, cc], nv[:, :w], zm[:, :w])
                    else:
                        hmn = gpool.tile([P, CH], BF16, tag="hmn", name="hmn")
                        nc.vector.tensor_sub(hmn[:, :w], h[:, cc], nv[:, :w])
                        zm = gpool.tile([P, CH], BF16, tag="zm", name="zm")
                        nc.vector.tensor_mul(zm[:, :w], z[:, :w], hmn[:, :w])
                        nc.vector.tensor_add(h[:, cc], zm[:, :w], nv[:, :w])

        # ---------------- pipeline driver ---------------------------------
        def key_of(item):
            t, g0, wg = item
            return (t, g0) if t != "d1" else ("d1", g0)

        raw = {}
        summed = {}
        for i in range(len(items) + 2):
            if i < len(items):
                raw[i] = gather4(key_of(items[i]))
            j = i - 1
            if 0 <= j < len(items):
                summed[j] = sub_sum(raw.pop(j), key_of(items[j]))
            k = i - 2
            if 0 <= k < len(items):
                consume(items[k], summed.pop(k))

        # ---------------- LayerNorm over D (partitions) for cols [0, N2) --
        for c0 in range(0, N2, CH):
            w = min(CH, N2 - c0)
            cc = slice(c0, c0 + w)
            pmu = ppool.tile([1, CH], F32, tag="pa", name="pmu")
            nc.tensor.matmul(pmu[:1, :w], oneP[:], h[:, cc], start=True, stop=True)
            sq = gpool.tile([P, CH], BF16, tag="r", name="sq")
            nc.vector.tensor_mul(sq[:, :w], h[:, cc], h[:, cc])
            ps2 = ppool.tile([1, CH], F32, tag="pb", name="ps2")
            nc.tensor.matmul(ps2[:1, :w], oneP[:], sq[:, :w], start=True, stop=True)
            mu = gpool.tile([1, CH], F32, tag="mu", name="mu")
            nc.scalar.copy(mu[:, :w], pmu[:1, :w])
            m2 = gpool.tile([1, CH], F32, tag="m2", name="m2")
            nc.vector.tensor_mul(m2[:, :w], mu[:, :w], mu[:, :w])
            var = gpool.tile([1, CH], F32, tag="var", name="var")
            nc.vector.tensor_sub(var[:, :w], ps2[:1, :w], m2[:, :w])
            std = gpool.tile([1, CH], F32, tag="std", name="std")
            nc.scalar.activation(std[:, :w], var[:, :w], AF.Sqrt, bias=eps_row[:])
            rstd = gpool.tile([1, CH], F32, tag="rstd", name="rstd")
            nc.vector.reciprocal(rstd[:, :w], std[:, :w])
            nmr = gpool.tile([1, CH], F32, tag="nmr", name="nmr")
            nc.vector.tensor_mul(nmr[:, :w], mu[:, :w], rstd[:, :w])
            pa = ppool.tile([P, CH], F32, tag="pc", name="pa_b")
            nc.tensor.matmul(pa[:, :w], ones_f[:1, :P], rstd[:1, :w],
                             start=True, stop=True)
            pb = ppool.tile([P, CH], F32, tag="pd", name="pb_b")
            nc.tensor.matmul(pb[:, :w], ones_f[:1, :P], nmr[:1, :w],
                             start=True, stop=True)
            tl = gpool.tile([P, CH], F32, tag="z", name="tl")
            nc.vector.tensor_mul(tl[:, :w], h[:, cc], pa[:, :w])
            t2 = gpool.tile([P, CH], F32, tag="nv", name="t2")
            nc.vector.tensor_sub(t2[:, :w], tl[:, :w], pb[:, :w])
            nc.vector.tensor_scalar(
                out=h[:, cc], in0=t2[:, :w], scalar1=gamma[:], scalar2=beta[:],
                op0=ALU.mult, op1=ALU.add)

        # ---------------- classifier: outT = wc @ ft + bc ------------------
        for c0 in range(0, Ncol, CH):
            w = min(CH, Ncol - c0)
            cc = slice(c0, c0 + w)
            pcl = ppool.tile([N_CLASSES, CH], F32, tag="pa", name="pcl")
            nc.tensor.matmul(pcl[:N_CLASSES, :w], wcT[:], h[:, cc],
                             start=True, stop=True)
            ot = opool.tile([N_CLASSES, CH], F32, tag="ot", name="ot")
            nc.scalar.activation(ot[:N_CLASSES, :w], pcl[:N_CLASSES, :w],
                                 AF.Identity, bias=bcc[:])
            nc.sync.dma_start(out=outT[:, cc], in_=ot[:N_CLASSES, :w])

    nc.compile()
    return nc


_PROGRAM_CACHE = {}


def _program(sched):
    key = (tuple(sched["n_t"]), sched["Ncol"], sched["kd1"])
    if key not in _PROGRAM_CACHE:
        _PROGRAM_CACHE[key] = _build_program(sched)
    return _PROGRAM_CACHE[key]


# ----------------------------------------------------------------- interface

def _in_maps(inputs):
    sched, idxbufs, colnode = _prep(
        inputs["token_id"], inputs["neigh_idx"], inputs["deg"])
    params = _params(
        inputs["emb"], inputs["w_ih"], inputs["w_hh"], inputs["b_ih"],
        inputs["b_hh"], inputs["gamma"], inputs["beta"], inputs["wc"],
        inputs["bc"])
    maps = [dict(params, idxbuf=idxbufs[c]) for c in range(NCORES)]
    return sched, maps, colnode


def _post(results, sched, colnode):
    out = np.zeros((N_DST, N_CLASSES), np.float32)
    for c in range(NCORES):
        oT = np.asarray(results[c]["outT"])  # [n_classes, Ncol]
        nid = colnode[c]
        valid = nid >= 0
        out[nid[valid]] = oT.T[valid]
    return out


def run(inputs, trace=False):
    sched, maps, colnode = _in_maps(inputs)
    nc = _program(sched)
    res = bass_utils.run_bass_kernel_spmd(
        nc, maps, core_ids=list(range(NCORES)), trace=trace)
    return _post(res.results, sched, colnode), res


def kernel(**inputs) -> np.ndarray:
    out, _ = run(inputs, trace=False)
    return out


# ------------------------------------------------- cached-jit timing harness

class TimedRunner:
    """Build the PJRT executable once; re-invoke on device-resident inputs."""

    def __init__(self, inputs):
        import jax
        from jax.sharding import Mesh, PartitionSpec
        from jax.experimental.shard_map import shard_map
        from concourse import bass2jax

        self.sched, maps, self.colnode = _in_maps(inputs)
        nc = _program(self.sched)
        bass2jax.install_neuronx_cc_hook()

        part_name = (nc.partition_id_tensor.name
                     if nc.partition_id_tensor else None)
        in_names, out_names, out_avals, zero_outs = [], [], [], []
        for alloc in nc.m.functions[0].allocations:
            if not isinstance(alloc, mybir.MemoryLocationSet):
                continue
            name = alloc.memorylocations[0].name
            if alloc.kind == "ExternalInput":
                if name != part_name:
                    in_names.append(name)
            elif alloc.kind == "ExternalOutput":
                out_names.append(name)
                dt_np = mybir.dt.np(alloc.dtype)
                out_avals.append(jax.core.ShapedArray(tuple(alloc.tensor_shape), dt_np))
                zero_outs.append(np.zeros(tuple(alloc.tensor_shape), dt_np))
        n_params = len(in_names)
        all_names = in_names + out_names
        if part_name is not None:
            all_names = all_names + [part_name]

        def _body(*args):
            operands = list(args)
            if part_name is not None:
                operands.append(bass2jax.partition_id_tensor())
            outs = bass2jax._bass_exec_p.bind(
                *operands,
                out_avals=tuple(out_avals),
                in_names=tuple(all_names),
                out_names=tuple(out_names),
                lowering_input_output_aliases=(),
                sim_require_finite=True,
                sim_require_nnan=True,
                nc=nc,
            )
            return tuple(outs)

        devices = jax.devices()[:NCORES]
        mesh = Mesh(np.asarray(devices), ("core",))
        nz = len(zero_outs)
        self.fn = jax.jit(
            shard_map(_body, mesh=mesh,
                      in_specs=(PartitionSpec("core"),) * (n_params + nz),
                      out_specs=(PartitionSpec("core"),) * nz,
                      check_rep=False),
            keep_unused=True)
        concat_in = [np.concatenate([np.asarray(m[n]) for m in maps], axis=0)
                     for n in in_names]
        self.dev_in = [jax.device_put(a) for a in concat_in]
        self.zero_outs = zero_outs
        # Device-resident output buffers: the kernel overwrites outT fully
        # every call, so reusing one buffer across calls is safe and avoids
        # re-uploading zeros through the tunnel on every invocation.
        self.dev_zeros = [
            jax.device_put(np.zeros((NCORES * z.shape[0], *z.shape[1:]), z.dtype))
            for z in zero_outs]
        self.out_names = out_names
        self.out_avals = out_avals
        self.jax = jax

    def __call__(self):
        outs = self.fn(*self.dev_in, *self.dev_zeros)
        self.jax.block_until_ready(outs)
        return outs

    def timed(self, iters=5):
        import time
        self()  # warm-up / compile
        times = []
        for _ in range(iters):
            t0 = time.perf_counter()
            self()
            times.append(time.perf_counter() - t0)
        return min(times)

    def result(self):
        outs = self()
        results = []
        for c in range(NCORES):
            d = {}
            for i, n in enumerate(self.out_names):
                full = np.asarray(outs[i])
                d[n] = full.reshape(NCORES, *self.out_avals[i].shape)[c]
            results.append(d)
        return _post(results, self.sched, self.colnode)
